# revision 9
# baseline (speedup 1.0000x reference)
"""Trainium2 Bass kernel for the ActivateLSDM module (LSTM + NeuralODE gate).

Strategy (8 NeuronCores, data-parallel over batch for the parallel phase):
  Phase 1 (batch-sharded, 8 rows/core): precompute, for every (b, t) row,
     G[row, :] = x[row] @ Wx + b_all  (gate order [f|i|o|c])
                 + [0, 0, ode(x[row]) @ wode, 0]   (into the o-gate slice)
     via big bf16 GEMMs in a feat-major (transposed) layout on the PE.
  Phase 2: sequential LSTM scan over t (per core, its own 8 batch rows):
     g_t = G[:, t, :] + h_t @ Wh ; gates; C/h update.

kernel(**inputs) takes FULL inputs, shards internally, returns FULL outputs
shaped like the reference: (hidden_seq, (hT, CT)).
"""

import os
import sys

sys.path.insert(0, "/opt/trn_rl_repo")

import numpy as np
import ml_dtypes

import concourse.bass as bass
import concourse.mybir as mybir
import concourse.bacc as bacc
import concourse.tile as tile
from concourse.bass_utils import run_bass_kernel_spmd

F32 = mybir.dt.float32
BF16 = mybir.dt.bfloat16
bf16 = ml_dtypes.bfloat16

# Problem dims
B, S, I, H = 64, 1024, 256, 512
G4 = 4 * H  # 2048 gate cols, order [f | i | o | c]
ODE_H = 2 * I  # 512
ODE_STEPS = 4
NCORES = 8
M = B // NCORES  # 8 batch rows per core
R = M * S  # 8192 rows per core
CHUNK = 512  # phase-1 row chunk
NCHUNK = R // CHUNK  # 16
UNROLL = 4  # scan steps per For_i iteration


def build(nc):
    """Build the full per-core kernel graph. Same graph on all 8 cores."""
    # ---------------- DRAM parameters ----------------
    x_d = nc.declare_dram_parameter("x", [R, I], F32, isOutput=False)
    h0_d = nc.declare_dram_parameter("h0", [M, H], F32, isOutput=False)
    c0_d = nc.declare_dram_parameter("c0", [M, H], F32, isOutput=False)
    wx_d = nc.declare_dram_parameter("wx", [I, G4], BF16, isOutput=False)
    wh_d = nc.declare_dram_parameter("wh", [H, G4], BF16, isOutput=False)
    ball_d = nc.declare_dram_parameter("ball", [1, G4], BF16, isOutput=False)
    a_d = nc.declare_dram_parameter("odeA", [I, ODE_H], BF16, isOutput=False)
    ba_d = nc.declare_dram_parameter("odeba", [ODE_H, 1], F32, isOutput=False)
    bd_d = nc.declare_dram_parameter("odeBd", [ODE_H, I], BF16, isOutput=False)
    wode_d = nc.declare_dram_parameter("wode", [I, ODE_H], BF16, isOutput=False)
    i8b_d = nc.declare_dram_parameter("i8b", [M, M], BF16, isOutput=False)
    i8f_d = nc.declare_dram_parameter("i8f", [M, M], F32, isOutput=False)

    outh_d = nc.declare_dram_parameter("out_h", [R, H], F32, isOutput=True)
    outc_d = nc.declare_dram_parameter("out_c", [M, H], F32, isOutput=True)

    # Internal DRAM: precomputed gate pre-activations, rows (b, t)
    g_dram = nc.dram_tensor("g_dram", [R, G4], BF16)

    with tile.TileContext(nc) as tc:
        # ---------------- constant weights in SBUF ----------------
        with tc.tile_pool(name="const", bufs=1) as cpool:
            # Wx as 2 k-tiles (128, 2048)
            wx_sb = [cpool.tile([128, G4], BF16, tag=f"wx{k}", name=f"wx{k}") for k in range(2)]
            for k in range(2):
                nc.sync.dma_start(wx_sb[k][:], wx_d.ap()[128 * k:128 * (k + 1), :])
            # Wh as 4 k-tiles (128, 2048)
            wh_sb = [cpool.tile([128, G4], BF16, tag=f"wh{k}", name=f"wh{k}") for k in range(4)]
            for k in range(4):
                nc.sync.dma_start(wh_sb[k][:], wh_d.ap()[128 * k:128 * (k + 1), :])
            # biases row (1, 2048)
            ball_sb = cpool.tile([1, G4], BF16, tag="ball", name="ball")
            nc.sync.dma_start(ball_sb[:], ball_d.ap()[:, :])
            # ODE A: 2 k-tiles (128, 512)
            a_sb = [cpool.tile([128, ODE_H], BF16, tag=f"a{k}", name=f"a{k}") for k in range(2)]
            for k in range(2):
                nc.sync.dma_start(a_sb[k][:], a_d.ap()[128 * k:128 * (k + 1), :])
            # ODE bias as per-partition cols (128, 4)
            ba_sb = cpool.tile([128, 4], F32, tag="ba", name="ba")
            nc.sync.dma_start(
                ba_sb[:], ba_d.ap().rearrange("(m p) one -> p (m one)", p=128)
            )
            # Bd (dt-scaled ode_B): 4 k-tiles (128, 256)
            bd_sb = [cpool.tile([128, I], BF16, tag=f"bd{k}", name=f"bd{k}") for k in range(4)]
            for k in range(4):
                nc.sync.dma_start(bd_sb[k][:], bd_d.ap()[128 * k:128 * (k + 1), :])
            # wode: 2 k-tiles (128, 512)
            wode_sb = [cpool.tile([128, ODE_H], BF16, tag=f"wo{k}", name=f"wo{k}") for k in range(2)]
            for k in range(2):
                nc.sync.dma_start(wode_sb[k][:], wode_d.ap()[128 * k:128 * (k + 1), :])
            # ones row for bias broadcast matmul
            ones_sb = cpool.tile([1, 128], BF16, tag="ones", name="ones")
            nc.gpsimd.memset(ones_sb[:], 1.0)
            # identities for the small scan transposes
            i8b_sb = cpool.tile([M, M], BF16, tag="i8b", name="i8b")
            nc.sync.dma_start(i8b_sb[:], i8b_d.ap()[:, :])
            i8f_sb = cpool.tile([M, M], F32, tag="i8f", name="i8f")
            nc.sync.dma_start(i8f_sb[:], i8f_d.ap()[:, :])

            # ================= PHASE 1 =================
            x2 = x_d.ap()  # (R, I)
            with (
                tc.tile_pool(name="p1", bufs=2) as p1,
                tc.tile_pool(name="p1w", bufs=1, space="PSUM") as psw,
                tc.tile_pool(name="p1p2", bufs=1, space="PSUM") as psp2,
            ):
                for c in range(NCHUNK):
                    r0 = c * CHUNK
                    # x chunk (512 rows, 256) -> sbuf (128, 4*256) bf16 (cast DMA)
                    xbf = p1.tile([128, 4 * I], BF16, tag="xbf", name="xbf")
                    nc.gpsimd.dma_start(
                        xbf[:],
                        x2[r0:r0 + CHUNK, :].rearrange("(s p) i -> p s i", p=128),
                    )
                    # transpose to feat-major xT (2 tiles of (128, 512 rows))
                    xt = [p1.tile([128, CHUNK], BF16, tag=f"xt{h}", name=f"xt{h}") for h in range(2)]
                    for s in range(4):
                        for h in range(2):
                            nc.sync.dma_start(
                                xt[h][:, 128 * s:128 * (s + 1)],
                                xbf[:, s * I + 128 * h: s * I + 128 * (h + 1)],
                                transpose=True,
                            )
                    # ODE: accumulate z-delta in psum across Euler steps
                    p2 = [psp2.tile([128, CHUNK], F32, tag=f"p2_{j}", name=f"p2_{j}") for j in range(2)]
                    zbf = [p1.tile([128, CHUNK], BF16, tag=f"zbf{j}", name=f"zbf{j}") for j in range(2)]
                    for st in range(ODE_STEPS):
                        zin = xt if st == 0 else zbf
                        ys = [psw.tile([128, CHUNK], F32, tag=f"w{m}", name=f"w{m}") for m in range(4)]
                        for m in range(4):
                            for k in range(2):
                                nc.tensor.matmul(
                                    ys[m][:],
                                    a_sb[k][:, 128 * m:128 * (m + 1)],
                                    zin[k][:],
                                    start=(k == 0),
                                    stop=(k == 1),
                                )
                        tbf = [p1.tile([128, CHUNK], BF16, tag=f"tbf{m}", name=f"tbf{m}") for m in range(4)]
                        for m in range(4):
                            nc.scalar.activation(
                                tbf[m][:], ys[m][:],
                                mybir.ActivationFunctionType.Tanh,
                                bias=ba_sb[:, m:m + 1], scale=1.0,
                            )
                        for j in range(2):
                            for m in range(4):
                                nc.tensor.matmul(
                                    p2[j][:],
                                    bd_sb[m][:, 128 * j:128 * (j + 1)],
                                    tbf[m][:],
                                    start=(st == 0 and m == 0),
                                    stop=(st == ODE_STEPS - 1 and m == 3),
                                )
                        # z_{st+1} = x + sum of deltas (psum) -> bf16
                        for j in range(2):
                            nc.vector.tensor_add(zbf[j][:], p2[j][:], xt[j][:])
                    # G chunk: per 128-row sub
                    for s in range(4):
                        gq = [psw.tile([128, 512], F32, tag=f"w{n}", name=f"w{n}") for n in range(4)]
                        for n in range(4):
                            # bias broadcast (K=1) first: clears psum
                            nc.tensor.matmul(
                                gq[n][:], ones_sb[:, 0:128],
                                ball_sb[:, 512 * n:512 * (n + 1)],
                                start=True, stop=False,
                            )
                            for k in range(2):
                                nc.tensor.matmul(
                                    gq[n][:],
                                    xt[k][:, 128 * s:128 * (s + 1)],
                                    wx_sb[k][:, 512 * n:512 * (n + 1)],
                                    start=False,
                                    stop=(k == 1 and n != 2),
                                )
                        # fold q = p @ wode into o-gate slice (cols 1024:1536 -> n=2)
                        for k in range(2):
                            nc.tensor.matmul(
                                gq[2][:],
                                zbf[k][:, 128 * s:128 * (s + 1)],
                                wode_sb[k][:],
                                start=False, stop=(k == 1),
                            )
                        gbf = p1.tile([128, G4], BF16, tag=f"gbf{s % 2}", name=f"gbf{s % 2}")
                        for n in range(4):
                            if n % 2 == 0:
                                nc.scalar.copy(gbf[:, 512 * n:512 * (n + 1)], gq[n][:])
                            else:
                                nc.vector.tensor_copy(gbf[:, 512 * n:512 * (n + 1)], gq[n][:])
                        nc.sync.dma_start(
                            g_dram.ap()[r0 + 128 * s: r0 + 128 * (s + 1), :], gbf[:]
                        )

            # ================= PHASE 2: scan =================
            g_v = g_dram.ap().rearrange("(b t) g -> b (t g)", b=M)  # (8, 1024*2048)
            h_v = outh_d.ap().rearrange("(b t) h -> b (t h)", b=M)  # (8, 1024*512)

            with (
                tc.tile_pool(name="state", bufs=1) as stp,
                tc.tile_pool(name="scan", bufs=2) as scp,
                tc.tile_pool(name="psg", bufs=1, space="PSUM") as psg,
                tc.tile_pool(name="psht", bufs=2, space="PSUM") as psht,
            ):
                c_sb = stp.tile([M, H], F32, tag="c_state", name="c_state")
                nc.sync.dma_start(c_sb[:], c0_d.ap()[:, :])
                ht_bf = stp.tile([128, 4 * M], BF16, tag="ht_state", name="ht_state")
                h0_sb = stp.tile([M, H], F32, tag="h0", name="h0")
                nc.sync.dma_start(h0_sb[:], h0_d.ap()[:, :])
                pst0 = psht.tile([128, 4 * M], F32, tag="psht", name="psht")
                for k in range(4):
                    nc.tensor.matmul(
                        pst0[:, M * k:M * (k + 1)],
                        h0_sb[:, 128 * k:128 * (k + 1)],
                        i8f_sb[:],
                        start=True, stop=True,
                    )
                nc.vector.tensor_copy(ht_bf[:], pst0[:])

                with tc.For_i(0, S // UNROLL, 1) as it:
                    u_sb = scp.tile([M, UNROLL * G4], BF16, tag="u", name="u")
                    nc.sync.dma_start(u_sb[:], g_v[:, bass.ds(it * (UNROLL * G4), UNROLL * G4)])
                    hseq = scp.tile([M, UNROLL * H], F32, tag="hseq", name="hseq")
                    for u in range(UNROLL):
                        ps = psg.tile([M, G4], F32, tag="ps", name="ps")
                        for n in range(4):
                            nc.tensor.matmul(
                                ps[:, 512 * n:512 * (n + 1)],
                                i8b_sb[:],
                                u_sb[:, u * G4 + 512 * n: u * G4 + 512 * (n + 1)],
                                start=True, stop=False,
                            )
                            for k in range(4):
                                nc.tensor.matmul(
                                    ps[:, 512 * n:512 * (n + 1)],
                                    ht_bf[:, M * k:M * (k + 1)],
                                    wh_sb[k][:, 512 * n:512 * (n + 1)],
                                    start=False, stop=(k == 3),
                                )
                        sfio = scp.tile([M, 3 * H], F32, tag="sfio", name="sfio")
                        nc.scalar.activation(
                            sfio[:], ps[:, 0:3 * H],
                            mybir.ActivationFunctionType.Sigmoid,
                        )
                        cc = scp.tile([M, H], F32, tag="cc", name="cc")
                        nc.scalar.activation(
                            cc[:], ps[:, 3 * H:4 * H],
                            mybir.ActivationFunctionType.Tanh,
                        )
                        t1 = scp.tile([M, H], F32, tag="t1", name="t1")
                        nc.vector.tensor_mul(t1[:], sfio[:, H:2 * H], cc[:])
                        t2 = scp.tile([M, H], F32, tag="t2", name="t2")
                        nc.vector.tensor_mul(t2[:], sfio[:, 0:H], c_sb[:])
                        nc.vector.tensor_add(c_sb[:], t1[:], t2[:])
                        thc = scp.tile([M, H], F32, tag="thc", name="thc")
                        nc.scalar.activation(
                            thc[:], c_sb[:], mybir.ActivationFunctionType.Tanh
                        )
                        hs = hseq[:, u * H:(u + 1) * H]
                        nc.vector.tensor_mul(hs, sfio[:, 2 * H:3 * H], thc[:])
                        # transpose h for next step
                        pst = psht.tile([128, 4 * M], F32, tag="psht", name="psht")
                        for k in range(4):
                            nc.tensor.matmul(
                                pst[:, M * k:M * (k + 1)],
                                hseq[:, u * H + 128 * k: u * H + 128 * (k + 1)],
                                i8f_sb[:],
                                start=True, stop=True,
                            )
                        nc.vector.tensor_copy(ht_bf[:], pst[:])
                    nc.sync.dma_start(
                        h_v[:, bass.ds(it * (UNROLL * H), UNROLL * H)], hseq[:]
                    )
                nc.sync.dma_start(outc_d.ap()[:, :], c_sb[:])
    return nc


_CACHED = {}


def run_timed(nc, in_maps, n_cores=NCORES, reps=3):
    """Like run_bass_via_pjrt but keeps inputs on device and times the
    executable: returns (results, best_exec_seconds)."""
    import time
    import jax
    from jax.experimental.shard_map import shard_map
    from jax.sharding import Mesh, PartitionSpec
    from concourse import bass2jax, mybir as _mb

    bass2jax.install_neuronx_cc_hook()
    partition_name = nc.partition_id_tensor.name if nc.partition_id_tensor else None
    in_names, out_names, out_avals, zero_outs = [], [], [], []
    for alloc in nc.m.functions[0].allocations:
        if not isinstance(alloc, _mb.MemoryLocationSet):
            continue
        name = alloc.memorylocations[0].name
        if alloc.kind == "ExternalInput":
            if name != partition_name:
                in_names.append(name)
        elif alloc.kind == "ExternalOutput":
            out_names.append(name)
            shape = tuple(alloc.tensor_shape)
            dtype = _mb.dt.np(alloc.dtype)
            out_avals.append(jax.core.ShapedArray(shape, dtype))
            zero_outs.append(np.zeros(shape, dtype))
    n_params = len(in_names)
    all_in_names = list(in_names) + list(out_names)
    if partition_name is not None:
        all_in_names.append(partition_name)

    def _body(*args):
        operands = list(args)
        if partition_name is not None:
            operands.append(bass2jax.partition_id_tensor())
        outs = bass2jax._bass_exec_p.bind(
            *operands,
            out_avals=tuple(out_avals),
            in_names=tuple(all_in_names),
            out_names=tuple(out_names),
            lowering_input_output_aliases=(),
            sim_require_finite=True,
            sim_require_nnan=True,
            nc=nc,
        )
        return tuple(outs)

    devices = jax.devices()[:n_cores]
    mesh = Mesh(np.asarray(devices), ("core",))
    n_outs = len(out_names)
    in_specs = (PartitionSpec("core"),) * (n_params + n_outs)
    out_specs = (PartitionSpec("core"),) * n_outs
    fn = jax.jit(
        shard_map(_body, mesh=mesh, in_specs=in_specs, out_specs=out_specs,
                  check_rep=False),
        keep_unused=True,
    )
    concat_in = [
        np.concatenate([np.asarray(in_maps[c][k]) for c in range(n_cores)], axis=0)
        for k in in_names
    ] + [np.concatenate([z] * n_cores, axis=0) for z in zero_outs]
    sharding = jax.sharding.NamedSharding(mesh, PartitionSpec("core"))
    dev_in = [jax.device_put(a, sharding) for a in concat_in]
    outs = fn(*dev_in)  # warmup/compile
    jax.block_until_ready(outs)
    best = None
    for _ in range(reps):
        t0 = time.perf_counter()
        o2 = fn(*dev_in)
        jax.block_until_ready(o2)
        dt = time.perf_counter() - t0
        best = dt if best is None else min(best, dt)
    results = []
    for c in range(n_cores):
        m = {}
        for i, name in enumerate(out_names):
            full = np.asarray(outs[i])
            per = full.shape[0] // n_cores
            m[name] = full[c * per:(c + 1) * per]
        results.append(m)
    return results, best


def _get_nc():
    if "nc" not in _CACHED:
        nc = bacc.Bacc(
            "TRN2",
            target_bir_lowering=False,
            debug=False,
            enable_asserts=False,
            num_devices=NCORES,
        )
        build(nc)
        nc.compile()
        _CACHED["nc"] = nc
    return _CACHED["nc"]


def kernel(x, h0, C0, Wf, bf, Wi, bi, Wc, bc, Wo, bo, wode, ode_A, ode_ba, ode_B,
           **kwargs):
    x = np.asarray(x, np.float32)
    h0 = np.asarray(h0, np.float32)
    C0 = np.asarray(C0, np.float32)
    # gate order [f | i | o | c]
    W_all = np.concatenate([np.asarray(Wf), np.asarray(Wi), np.asarray(Wo),
                            np.asarray(Wc)], axis=1).astype(np.float32)
    b_all = np.concatenate([np.asarray(bf), np.asarray(bi), np.asarray(bo),
                            np.asarray(bc)], axis=0).astype(np.float32)
    Wh = np.ascontiguousarray(W_all[:H, :]).astype(bf16)
    Wx = np.ascontiguousarray(W_all[H:, :]).astype(bf16)
    ball = b_all.reshape(1, G4).astype(bf16)
    A = np.asarray(ode_A, np.float32).astype(bf16)
    ba = np.asarray(ode_ba, np.float32).reshape(ODE_H, 1)
    Bd = (np.asarray(ode_B, np.float32) / ODE_STEPS).astype(bf16)
    wode_b = np.asarray(wode, np.float32).astype(bf16)
    i8b = np.eye(M, dtype=np.float32).astype(bf16)
    i8f = np.eye(M, dtype=np.float32)

    nc = _get_nc()
    in_maps = []
    for c in range(NCORES):
        xs = np.ascontiguousarray(x[c * M:(c + 1) * M].reshape(R, I))
        in_maps.append({
            "x": xs,
            "h0": np.ascontiguousarray(h0[c * M:(c + 1) * M]),
            "c0": np.ascontiguousarray(C0[c * M:(c + 1) * M]),
            "wx": Wx, "wh": Wh, "ball": ball,
            "odeA": A, "odeba": ba, "odeBd": Bd, "wode": wode_b,
            "i8b": i8b, "i8f": i8f,
        })

    timed = os.environ.get("BASS_KERNEL_TIME", "0") == "1"
    if timed:
        results, best = run_timed(nc, in_maps)
        _CACHED["exec_time_ns"] = int(best * 1e9)
        print(f"HW exec time: {int(best * 1e9)} ns", file=sys.stderr)
    else:
        res = run_bass_kernel_spmd(nc, in_maps, core_ids=list(range(NCORES)))
        results = res.results

    hidden = np.concatenate(
        [results[c]["out_h"].reshape(M, S, H) for c in range(NCORES)], axis=0
    )
    CT = np.concatenate([results[c]["out_c"] for c in range(NCORES)], axis=0)
    hT = np.ascontiguousarray(hidden[:, -1, :])
    return hidden, (hT, CT)


if __name__ == "__main__":
    # smoke: random small check against numpy reference
    pass
